# revision 8
# baseline (speedup 1.0000x reference)
"""Trainium2 Bass kernel for byte-to-patch cross attention (v3).

Problem shapes (hardcoded): B=2, S=4096, P=1024, D=1024, H=16 heads, dh=64.

Sharding: 8 cores = batch (2) x head-groups (4). Core i handles batch i//4
and heads 4*(i%4) .. 4*(i%4)+3 (a 256-wide slice of the projection dims).
The host sums the 4 partial output projections per batch and adds the
bv/bo bias terms.

All matmuls are fp16 (fp8 fails the 2e-2 accuracy gate). Structural
optimizations vs the original baseline:
 - The softmax denominator is folded into the PV matmul: the stationary
   operand per (patch-tile, head) is [V_h (64 cols) | ones (64 dup cols)],
   so psum rows 64..127 hold the denominator already broadcast across 64
   partitions at no extra cost (matmul time depends only on the moving
   free size).  This deletes the baseline's separate ones-matmuls.
 - Normalization is one reciprocal (psum rows 64.. -> sbuf) and one
   cross-partition multiply per head.
 - Scores psum tiles span 2 banks (both heads of a pair), so one ACT exp
   instruction covers ~1024 columns, amortizing the fixed access cost.
 - Boundary tiles restrict scores/exp/PV to the visible byte range
   [t0, 512), and a host-precomputed 0/1 mask is multiplied only over the
   boundary band [t0, t1) on the otherwise idle GPSIMD engine.
 - Dedicated psum pools (scores / pv / out-proj) and interleaved emission
   keep the PE from stalling on ACT/DVE round trips.
"""

import sys

sys.path.insert(0, "/opt/trn_rl_repo")

import numpy as np

import concourse.bass as bass
import concourse.mybir as mybir
from concourse import bacc, tile
from concourse.bass_utils import run_bass_kernel_spmd

B, S, P, D, H = 2, 4096, 1024, 1024, 16
HPC = H // 4          # heads per core = 4
GD = HPC * 64         # projection dim slice per core = 256
DH = 64               # head dim
SCALE = 1.0 / 8.0     # 1/sqrt(dh), folded into the exp's input scale

F16 = mybir.dt.float16
F32 = mybir.dt.float32

TC = 512              # byte-seq chunk (psum bank free size)
NTC = S // TC         # 8
NPT = P // 128        # 8 patch tiles
NK = D // 128         # 8 contraction chunks for fp16 projections

_CACHE = {}


def _build_program(plan):
    """plan: list over tc of list over pt of (vis, t0, t1); vis 0=skip,
    2=full, 1=boundary (mask band [t0,t1), live bytes [t0,512))."""
    nc = bacc.Bacc("TRN2", target_bir_lowering=False, debug=False)

    nmask = sum(1 for row in plan for v, _, _ in row if v == 1)

    xt_d = nc.dram_tensor("xt", [128, 8 * S], F16, kind="ExternalInput")
    rt_d = nc.dram_tensor("rt", [128, 8 * P], F16, kind="ExternalInput")
    wq_d = nc.dram_tensor("wq", [128, 8 * GD], F16, kind="ExternalInput")
    wk_d = nc.dram_tensor("wk", [128, 8 * GD], F16, kind="ExternalInput")
    wv_d = nc.dram_tensor("wv", [128, 8 * GD], F16, kind="ExternalInput")
    wo_d = nc.dram_tensor("wo", [128, 2 * D], F16, kind="ExternalInput")
    msk_d = nc.dram_tensor("msk", [128, max(nmask, 1) * TC], F16,
                           kind="ExternalInput")
    bqc_d = nc.dram_tensor("bqc", [128, 2], F32, kind="ExternalInput")
    bkc_d = nc.dram_tensor("bkc", [128, 2], F32, kind="ExternalInput")
    y_d = nc.dram_tensor("y", [S, D], F16, kind="ExternalOutput")

    with tile.TileContext(nc) as tc:
        with (
            tc.tile_pool(name="const", bufs=1) as cpool,
            tc.tile_pool(name="xt", bufs=1) as xt_pool,
            tc.tile_pool(name="qk", bufs=1) as qk_pool,
            tc.tile_pool(name="vt", bufs=1) as vt_pool,
            tc.tile_pool(name="probs", bufs=10) as pr_pool,
            tc.tile_pool(name="ot", bufs=2) as ot_pool,
            tc.tile_pool(name="rc", bufs=4) as rc_pool,
            tc.tile_pool(name="yout", bufs=4) as y_pool,
            tc.tile_pool(name="ps_sc", bufs=2, space="PSUM") as ps_sc,
            tc.tile_pool(name="ps_pv", bufs=2, space="PSUM") as ps_pv,
            tc.tile_pool(name="ps_op", bufs=1, space="PSUM") as ps_op,
        ):
            # ---- constant / weight loads --------------------------------
            wk = cpool.tile([128, 8 * GD], F16, tag="wk")
            nc.sync.dma_start(wk[:], wk_d[:, :])
            wqw = cpool.tile([128, 8 * GD], F16, tag="wqw")
            nc.sync.dma_start(wqw[:], wq_d[:, :])
            rt = cpool.tile([128, 8 * P], F16, tag="rt")
            nc.sync.dma_start(rt[:], rt_d[:, :])
            wv = cpool.tile([128, 8 * GD], F16, tag="wv")
            nc.sync.dma_start(wv[:], wv_d[:, :])
            wo = cpool.tile([128, 2 * D], F16, tag="wo")
            nc.sync.dma_start(wo[:], wo_d[:, :])
            bqc = cpool.tile([128, 2], F32, tag="bqc")
            nc.sync.dma_start(bqc[:], bqc_d[:])
            bkc = cpool.tile([128, 2], F32, tag="bkc")
            nc.sync.dma_start(bkc[:], bkc_d[:])
            msk = cpool.tile([128, max(nmask, 1) * TC], F16, tag="msk")
            nc.sync.dma_start(msk[:], msk_d[:, :])

            # ---- K^T projection -----------------------------------------
            # kt [128, 2, P]: m-tile m holds heads (2m, 2m+1) on row halves.
            kt = qk_pool.tile([128, 2, P], F16, name="kt")
            for m in range(2):
                pkp = ps_sc.tile([128, 1024], F32, tag="sc")
                for pc in range(2):
                    for k in range(NK):
                        nc.tensor.matmul(
                            pkp[:, pc * 512:(pc + 1) * 512],
                            wk[:, k * GD + m * 128:k * GD + (m + 1) * 128],
                            rt[:, k * P + pc * 512:k * P + (pc + 1) * 512],
                            start=(k == 0), stop=(k == NK - 1),
                        )
                nc.vector.tensor_scalar_add(
                    kt[:, m, :], pkp[:].rearrange("p (k c) -> p k c", k=2),
                    bkc[:, m:m + 1])

            # ---- V projection + broadcast ones columns ------------------
            # vt [128, (pt:8), (h:4), 128]: 64 V cols then 64 dup ones cols,
            # so the PV matmul lands the denominator broadcast across psum
            # rows 64..127 at no extra cost (M does not affect matmul time).
            vt = vt_pool.tile([128, 8, 4, 128], F16, name="vt")
            nc.vector.memset(vt[:, :, :, 64:128], 1.0)
            for half in range(2):
                pvp = ps_op.tile([128, 1024], F32, tag="op")
                for q in range(4):
                    pt = half * 4 + q
                    for k in range(NK):
                        nc.tensor.matmul(
                            pvp[:, q * 256:(q + 1) * 256],
                            rt[:, k * P + pt * 128:k * P + (pt + 1) * 128],
                            wv[:, k * GD:(k + 1) * GD],
                            start=(k == 0), stop=(k == NK - 1),
                        )
                nc.vector.tensor_copy(
                    vt[:, half * 4:(half + 1) * 4, :, 0:64],
                    pvp[:].rearrange("p (q h c) -> p q h c", q=4, h=4))

            # ---- Q^T projection -----------------------------------------
            xt = xt_pool.tile([128, 8, S], F16, name="xt")
            nc.sync.dma_start(xt[:].rearrange("p k t -> p (k t)"), xt_d[:, :])
            qt = qk_pool.tile([128, 2, S], F16, name="qt")
            for m in range(2):
                for th in range(4):
                    pqp = ps_sc.tile([128, 1024], F32, tag="sc")
                    for cc in range(2):
                        t0 = th * 1024 + cc * 512
                        for k in range(NK):
                            nc.tensor.matmul(
                                pqp[:, cc * 512:(cc + 1) * 512],
                                wqw[:, k * GD + m * 128:k * GD + (m + 1) * 128],
                                xt[:, k, t0:t0 + 512],
                                start=(k == 0), stop=(k == NK - 1),
                            )
                    nc.vector.tensor_scalar_add(
                        qt[:, m, th * 1024:(th + 1) * 1024],
                        pqp[:].rearrange("p (k c) -> p k c", k=2),
                        bqc[:, m:m + 1])

            # ---- attention + output projection, per byte-chunk ----------
            mask_idx = {}
            mi = 0
            for tci in range(NTC):
                for pt in range(NPT):
                    if plan[tci][pt][0] == 1:
                        mask_idx[(tci, pt)] = mi
                        mi += 1

            ot_tiles = [ot_pool.tile([128, S], F16, name="ot_t")
                        for _ in range(2)]

            for tci in range(NTC):
                tbase = tci * TC
                live = [pt for pt in range(NPT) if plan[tci][pt][0] > 0]
                pr_tiles = {}

                def emit_scores(g2, pt):
                    vis, t0, t1 = plan[tci][pt]
                    if vis == 2:
                        t0 = 0
                    psc = ps_sc.tile([128, 1024], F32, name="psc", tag="sc")
                    pr = pr_tiles[pt]
                    for hh in range(2):
                        nc.tensor.matmul(
                            psc[:, hh * 512 + t0:(hh + 1) * 512],
                            kt[64 * hh:64 * hh + 64, g2,
                               pt * 128:(pt + 1) * 128],
                            qt[64 * hh:64 * hh + 64, g2,
                               tbase + t0:tbase + TC],
                            start=True, stop=True,
                            tile_position=(64 * hh, 0),
                        )
                    nc.scalar.activation(
                        pr[:, 2 * g2:2 * g2 + 2, t0:TC],
                        psc[:].rearrange("p (k c) -> p k c", k=2)[:, :, t0:TC],
                        mybir.ActivationFunctionType.Exp, scale=SCALE,
                    )
                    if vis == 1 and t1 > t0:
                        i = mask_idx[(tci, pt)]
                        for hh in range(2):
                            h = 2 * g2 + hh
                            nc.gpsimd.tensor_tensor(
                                pr[:, h, t0:t1], pr[:, h, t0:t1],
                                msk[:, i * TC + t0:i * TC + t1],
                                op=mybir.AluOpType.mult,
                            )

                def emit_pv(g2, pt, pvh, last):
                    vis, t0, t1 = plan[tci][pt]
                    if vis == 2:
                        t0 = 0
                    for hh in range(2):
                        h = 2 * g2 + hh
                        nc.tensor.matmul(
                            pvh[hh][:, t0:TC],
                            vt[:, pt, h, :],
                            pr_tiles[pt][:, h, t0:TC],
                            start=(pt == live[0]), stop=last,
                            skip_group_check=True,
                        )

                for g2 in range(2):
                    if g2 == 0:
                        for pt in live:
                            pr_tiles[pt] = pr_pool.tile(
                                [128, 4, TC], F16, name="pr", tag="pr")
                    pvh = [ps_pv.tile([128, TC], F32, name="pv", tag="pv")
                           for _ in range(2)]
                    # pipelined: scores(j) ... pv(j-1) so the PE has work
                    # while ACT exps the previous tile.
                    for j, pt in enumerate(live):
                        emit_scores(g2, pt)
                        if j >= 1:
                            emit_pv(g2, live[j - 1], pvh, last=False)
                    emit_pv(g2, live[-1], pvh, last=True)

                    # normalize: recip of broadcast den rows, then multiply
                    for hh in range(2):
                        rc = rc_pool.tile([64, TC], F16, name="rc", tag="rc")
                        with nc.allow_low_precision(
                                reason="softmax 1/den in f16"):
                            nc.vector.reciprocal(rc[:], pvh[hh][64:128, :])
                        nc.vector.tensor_tensor(
                            ot_tiles[g2][64 * hh:64 * hh + 64,
                                         tbase:tbase + TC],
                            pvh[hh][0:64, :], rc[:],
                            op=mybir.AluOpType.mult,
                        )

                # output projection for this byte chunk
                for tt in range(4):
                    t0 = tbase + tt * 128
                    py = ps_op.tile([128, 1024], F32, name="py", tag="op")
                    for n in range(2):
                        for k2 in range(2):
                            nc.tensor.matmul(
                                py[:, n * 512:(n + 1) * 512],
                                ot_tiles[k2][:, t0:t0 + 128],
                                wo[:, k2 * D + n * 512:k2 * D + (n + 1) * 512],
                                start=(k2 == 0), stop=(k2 == 1),
                            )
                    ysb = y_pool.tile([128, D], F16, name="ysb", tag="y")
                    nc.vector.tensor_copy(ysb[:], py[:])
                    nc.sync.dma_start(y_d[t0:t0 + 128, :], ysb[:])

    nc.compile()
    return nc


def _plan(patch_boundaries):
    """Per (tc, pt): (vis, t0, t1). vis: 0 = skip (masked in every batch),
    2 = fully visible everywhere, 1 = boundary.  t0 = first in-chunk byte
    that can see this patch tile (min over batches), t1 = first byte from
    which the tile is fully visible in all batches."""
    cs = np.cumsum(patch_boundaries, axis=1)  # [B, S]
    plan = []
    for tci in range(NTC):
        seg = cs[:, tci * TC:(tci + 1) * TC]
        lo = int(seg[:, 0].min())
        hi = int(seg[:, -1].max())
        row = []
        for pt in range(NPT):
            if pt * 128 > hi:
                row.append((0, 0, 0))
            elif (pt + 1) * 128 - 1 <= lo:
                row.append((2, 0, 0))
            else:
                vis = np.argmax(seg >= pt * 128, axis=1)      # per batch
                t0 = int(min(vis[b] if seg[b, -1] >= pt * 128 else TC
                             for b in range(B)))
                full = np.argmax(seg >= (pt + 1) * 128 - 1, axis=1)
                t1 = int(max(full[b] if seg[b, -1] >= (pt + 1) * 128 - 1
                             else TC for b in range(B)))
                row.append((1, t0, t1))
        plan.append(row)
    return plan


def _get_program(plan):
    key = tuple(tuple(r) for r in plan)
    if key not in _CACHE:
        _CACHE[key] = _build_program(plan)
    return _CACHE[key]


def _prep_inputs(queries, patch_representations, patch_boundaries,
                 wq, wk, wv, wo, bq, bk, plan):
    def blk(a, n):   # [D, C] -> [128, n*C] with k-chunks side by side
        C = a.shape[1]
        return np.ascontiguousarray(
            a.reshape(n, 128, C).transpose(1, 0, 2).reshape(128, n * C)
        ).astype(np.float16)

    in_maps = []
    for core in range(8):
        b, g = core // 4, core % 4
        sl = slice(g * GD, (g + 1) * GD)
        xt = queries[b].T                             # [D, S]
        rt = patch_representations[b].T               # [D, P]
        cs = np.cumsum(patch_boundaries[b])
        masks = []
        for tci in range(NTC):
            for pt in range(NPT):
                vis, t0, t1 = plan[tci][pt]
                if vis != 1:
                    continue
                j = pt * 128 + np.arange(128)
                c = cs[tci * TC:(tci + 1) * TC]
                masks.append((j[:, None] <= c[None, :]).astype(np.float16))
        msk = (np.concatenate(masks, axis=1) if masks
               else np.zeros((128, TC), np.float16))
        wot = wo[:, sl].T.reshape(2, 128, D).transpose(1, 0, 2)
        wot = np.ascontiguousarray(wot.reshape(128, 2 * D)).astype(np.float16)
        in_maps.append({
            "xt": blk(xt, 8),
            "rt": blk(rt, 8),
            "wq": blk(wq[sl, :].T, 8),
            "wk": blk(wk[sl, :].T, 8),
            "wv": blk(wv[sl, :].T, 8),
            "wo": wot,
            "msk": np.ascontiguousarray(msk),
            "bqc": np.ascontiguousarray(
                bq[sl].reshape(2, 128).T).astype(np.float32),
            "bkc": np.ascontiguousarray(
                bk[sl].reshape(2, 128).T).astype(np.float32),
        })
    return in_maps


def _reduce_outputs(results, wo, bv, bo):
    y = np.zeros((B, S, D), dtype=np.float32)
    for core in range(8):
        y[core // 4] += results[core]["y"].astype(np.float32)
    y += (bv @ wo.T + bo)[None, None, :]
    return y


def kernel(queries, patch_representations, patch_boundaries,
           wq, wk, wv, wo, bq, bk, bv, bo):
    queries = np.asarray(queries, dtype=np.float32)
    patch_representations = np.asarray(patch_representations, dtype=np.float32)
    patch_boundaries = np.asarray(patch_boundaries)
    wq, wk, wv, wo = (np.asarray(a, dtype=np.float32) for a in (wq, wk, wv, wo))
    bq, bk, bv, bo = (np.asarray(a, dtype=np.float32) for a in (bq, bk, bv, bo))
    plan = _plan(patch_boundaries)
    nc = _get_program(plan)
    in_maps = _prep_inputs(queries, patch_representations, patch_boundaries,
                           wq, wk, wv, wo, bq, bk, plan)
    res = run_bass_kernel_spmd(nc, in_maps, core_ids=list(range(8)))
    return _reduce_outputs(res.results, wo, bv, bo)
